# revision 29
# baseline (speedup 1.0000x reference)
"""Trainium2 Bass kernel for Mistral-style MHA prefill (sparse_attention).

Problem: B=2, S=2048, DIM=4096, 32 q heads / 8 kv heads, head_dim=128,
sliding window 2048 (== S, pure causal), RoPE, fp32 reference.

Sharding (8 cores): data-parallel over batch (2) x tensor-parallel over heads
(4).  Core c = b*4 + tp handles batch b, q-heads [tp*8, tp*8+8), kv-heads
[tp*2, tp*2+2).  wq/wk/wv sharded column-wise, wo row-wise; the all-reduce
after wo is done on the host (f32 sum of the 4 bf16 partials per batch).

v2 dataflow ("all-transposed attention", zero PE transposes):
  - Q^T/K^T produced as [dh, s] (lhsT=w chunk, rhs=x^T chunk); V natural
    [s, dh] (lhsT=x^T chunk, rhs=wv).  RoPE via pair-swap permutation matmul
    + 3 DVE ops (as in v1).
  - scores computed TRANSPOSED: S^T[k, q-chunk512] = matmul(lhsT=K^T 128-col
    block, rhs=Q^T chunk).  Causal: k-blocks <= q-block only; the 4 diagonal
    blocks are column-restricted with a transposed 128x128 triangular mask.
  - P^T = exp(scale * S^T) on the scalar engine into a ring of [128,512]
    bf16 slices; invalid prefixes of diagonal slices zeroed by gpsimd memset.
  - PV transposed: A^T[dh, q] += matmul(lhsT=V[kb-block, dh], rhs=P^T slice)
    -- V natural is the stationary operand, so NO P/A transposes at all and
    A^T is directly the lhsT for the wo GEMM.
  - softmax row sums (= partition-dim sums of P^T): DVE accumulates slices
    into two bf16 chains, one PSUM-accumulated all-ones matmul does the
    partition sum AND the broadcast, DVE reciprocal, and the normalization
    is fused into the PSUM->SBUF copy of A^T (tensor_mul by linv).
  - wo: out[s-block, e] = sum_hb A^T[hb].T @ woT[hb] with woT streamed per
    512-e-column piece; output partials written bf16.
  - Scheduling: per half -> per 512-q-chunk.  exp on scalar is ~1.6x slower
    than the attention matmuls, so wo(qc-1) groups and next-phase projection
    groups are explicitly interleaved into the attention emission as PE
    filler work.  Two DMA queues (sync: x/wq/out, scalar: tables/wkv/wo)
    with weights-first ordering to minimize the startup stall.
"""

import os
import sys

import numpy as np

for _p in ("/opt/trn_rl_repo",):
    if _p not in sys.path and os.path.isdir(_p):
        sys.path.insert(0, _p)

import ml_dtypes  # noqa: E402

import concourse.bass as bass  # noqa: E402
import concourse.bass_isa as bass_isa  # noqa: E402
import concourse.mybir as mybir  # noqa: E402
import concourse.tile as tile  # noqa: E402
from concourse.bass_utils import run_bass_kernel_spmd  # noqa: E402

BF16 = ml_dtypes.bfloat16


def _install_drain_split_patch():
    """The pinned walrus rejects Tile's kernel-tail Drain when it carries more
    than ~2 semaphore waits ("Too many sync wait commands").  Split the global
    drain's waits across trailing sync-engine nops (1 wait each)."""
    if getattr(tile.TileContext, "_drain_split_patched", False):
        return
    from concourse.vector_clock import ScopedClock

    limit = 1

    def _patched_dab(self, tick_clock, wait_clock):
        drain_inst = self.nc.sync.drain()
        raw = drain_inst.ins
        wait_clock.add_sem_waits(raw, ScopedClock({None: tick_clock.global_clock}))
        si = raw.sync_info
        waits = list(si.on_wait or [])
        if len(waits) > limit:
            si.on_wait = waits[:limit]
            for i in range(limit, len(waits), limit):
                nraw = self.nc.sync.nop().ins
                nsi = nraw.sync_info
                if nsi is None:
                    nraw.sync_info = mybir.SyncInfo(
                        on_wait=waits[i : i + limit], on_update=[]
                    )
                else:
                    nsi.on_wait = list(nsi.on_wait or []) + waits[i : i + limit]
        self.nc.all_engine_barrier()
        popped = self.nc._tile_sem_poison_stack.pop()
        assert popped is self._sem_poison
        self.nc.clear_and_free_semaphores(list(self.sems.allocated().values()))
        self.nc.all_engine_barrier()

    tile.TileContext._drain_and_barrier = _patched_dab
    tile.TileContext._drain_split_patched = True


_install_drain_split_patch()

P = 128
S = 2048
D = 4096
KO = D // P  # 32 contraction chunks
NH_L = 8  # q heads per core
NKV_L = 2  # kv heads per core
DH = 128
SCALE = float(DH) ** -0.5
N_CORES = 8

_dt_f32 = mybir.dt.float32
_dt_bf16 = mybir.dt.bfloat16
_EXP = mybir.ActivationFunctionType.Exp


def _emit(tc, aps):
    nc = tc.nc
    xr = aps["xT"].rearrange("(ko p) s -> p ko s", p=P)  # [128, 32, 2048]
    wqr = aps["wqT"].rearrange("(ko p) o -> p ko o", p=P)  # [128, 32, 1024]
    wkvr = aps["wkvT"].rearrange("(ko p) o -> p ko o", p=P)  # [128, 32, 512]
    wor = aps["woT"].rearrange("(ho p) e -> p ho e", p=P)  # [128, 8, 4096]
    out_ap = aps["out"]  # [2048, 4096] bf16
    cexp_ap = aps["cexp"]  # [128, 2048] bf16
    sexp_ap = aps["sexp"]

    from contextlib import ExitStack

    with ExitStack() as g:
        consts = g.enter_context(tc.tile_pool(name="consts", bufs=1))
        tabs = g.enter_context(tc.tile_pool(name="tabs", bufs=1))
        kt_pool = g.enter_context(tc.tile_pool(name="ktp", bufs=1))
        vt_pool = g.enter_context(tc.tile_pool(name="vtp", bufs=1))
        xt_pool = g.enter_context(tc.tile_pool(name="xtp", bufs=1))
        qt_pool = g.enter_context(tc.tile_pool(name="qtp", bufs=1))
        wq_pool = g.enter_context(tc.tile_pool(name="wqp", bufs=3))
        wkv_pool = g.enter_context(tc.tile_pool(name="wkvp", bufs=3))
        pt_pool = g.enter_context(tc.tile_pool(name="ptp", bufs=16))
        at_pool = g.enter_context(tc.tile_pool(name="atp", bufs=2))
        wo_pool = g.enter_context(tc.tile_pool(name="wop", bufs=2))
        l_pool = g.enter_context(tc.tile_pool(name="lp", bufs=1))
        ost_pool = g.enter_context(tc.tile_pool(name="ostp", bufs=2))
        tmp_pool = g.enter_context(tc.tile_pool(name="tmpp", bufs=2))
        # PSUM: 3 + 1 + 2 + 2 = 8 banks
        ps_sc = g.enter_context(tc.tile_pool(name="ps_sc", bufs=3, space="PSUM"))
        ps_sw = g.enter_context(tc.tile_pool(name="ps_sw", bufs=1, space="PSUM"))
        ps_acc = g.enter_context(tc.tile_pool(name="ps_acc", bufs=2, space="PSUM"))
        ps_pa = g.enter_context(tc.tile_pool(name="ps_pa", bufs=2, space="PSUM"))

        perm_t = consts.tile([P, P], _dt_bf16)
        maskT_t = consts.tile([P, P], _dt_f32)
        ones_t = consts.tile([P, P], _dt_bf16)
        nc.scalar.dma_start(out=perm_t, in_=aps["perm"])
        nc.scalar.dma_start(out=maskT_t, in_=aps["maskT"])
        nc.scalar.dma_start(out=ones_t, in_=aps["ones"])

        for wi in range(6):
            wup = ps_sc.tile([P, 512], _dt_f32, tag="sc")
            nc.tensor.matmul(
                wup[:, :P], lhsT=perm_t, rhs=perm_t, start=True, stop=True
            )

        kt_t = kt_pool.tile([P, NKV_L, S], _dt_bf16)  # K^T roped, full S
        v_t = vt_pool.tile([P, 16, NKV_L * DH], _dt_bf16)  # V natural, full S

        cp_flip = [0]

        def cp(out, in_):
            if cp_flip[0] % 2 == 0:
                nc.scalar.copy(out=out, in_=in_)
            else:
                nc.vector.tensor_copy(out=out, in_=in_)
            cp_flip[0] += 1

        # ---- filler plumbing: closures emitting ~1 PE accumulation group ----
        fillers = []

        def drain(n):
            k = min(n, len(fillers))
            for _ in range(k):
                fillers.pop(0)()

        def drain_all():
            drain(len(fillers))

        def rope(dst, gsrc, ctab_t, stab_t, sc):
            """dst (sbuf bf16 [128,512]) <- rope(gsrc (psum f32 [128,512]))."""
            nc.scalar.copy(out=dst, in_=gsrc)
            sw = ps_sw.tile([P, 512], _dt_f32, tag="sw")
            nc.tensor.matmul(sw, lhsT=perm_t, rhs=dst, start=True, stop=True)
            t1 = tmp_pool.tile([P, 512], _dt_bf16, tag="t1")
            nc.vector.tensor_mul(t1, sw, stab_t[:, sc * 512 : (sc + 1) * 512])
            nc.vector.tensor_mul(dst, dst, ctab_t[:, sc * 512 : (sc + 1) * 512])
            nc.vector.tensor_add(dst, dst, t1)

        def v_group(xt_t, hi, sb_l, va, vb):
            sb = hi * 8 + sb_l
            gv = ps_acc.tile([P, 512], _dt_f32, tag="acc")
            for ko in range(KO):
                w = va if ko < 16 else vb
                nc.tensor.matmul(
                    gv[:, :256],
                    lhsT=xt_t[:, ko, sb_l * P : (sb_l + 1) * P],
                    rhs=w[:, ko % 16, :],
                    start=(ko == 0),
                    stop=(ko == KO - 1),
                )
            cp(v_t[:, sb, :], gv[:, :256])

        def k_group(xt_t, hi, sc, g2, ka, kb_, ctab_t, stab_t):
            gk = ps_acc.tile([P, 512], _dt_f32, tag="acc")
            for ko in range(KO):
                w = ka if ko < 16 else kb_
                nc.tensor.matmul(
                    gk,
                    lhsT=w[:, ko % 16, g2 * P : (g2 + 1) * P],
                    rhs=xt_t[:, ko, sc * 512 : (sc + 1) * 512],
                    start=(ko == 0),
                    stop=(ko == KO - 1),
                )
            rope(
                kt_t[:, g2, hi * 1024 + sc * 512 : hi * 1024 + (sc + 1) * 512],
                gk, ctab_t, stab_t, sc,
            )

        def q_group(xt_t, qt_t, hp, h2, sc, wa, wb, ctab_t, stab_t, rev=False):
            gq = ps_acc.tile([P, 512], _dt_f32, tag="acc")
            kos = range(KO - 1, -1, -1) if rev else range(KO)
            first = KO - 1 if rev else 0
            last = 0 if rev else KO - 1
            for ko in kos:
                w = wa if ko < 16 else wb
                nc.tensor.matmul(
                    gq,
                    lhsT=w[:, ko % 16, h2 * P : (h2 + 1) * P],
                    rhs=xt_t[:, ko, sc * 512 : (sc + 1) * 512],
                    start=(ko == first),
                    stop=(ko == last),
                )
            rope(
                qt_t[:, hp * 2 + h2, sc * 512 : (sc + 1) * 512],
                gq, ctab_t, stab_t, sc,
            )

        def attention(qc, qt_t):
            """S^T attention for 512-q chunk qc (global 0..3), all 8 heads.
            Fillers drained inside the kb loops to cover the exp deficit."""
            qc_l = qc & 1
            at_t = at_pool.tile([P, NH_L, 512], _dt_bf16, tag="at")
            nblk = 4 * qc + 4
            for h in range(NH_L):
                g2 = h // 4
                la = l_pool.tile([P, 512], _dt_bf16, tag="lacc_a")
                lb = l_pool.tile([P, 512], _dt_bf16, tag="lacc_b")
                slices = []
                for kb in range(nblk):
                    j = kb - 4 * qc
                    c0 = j * P if j > 0 else 0
                    ssc = ps_sc.tile([P, 512], _dt_f32, tag="sc")
                    nc.tensor.matmul(
                        ssc[:, c0:512],
                        lhsT=kt_t[:, g2, kb * P : (kb + 1) * P],
                        rhs=qt_t[:, h, qc_l * 512 + c0 : qc_l * 512 + 512],
                        start=True,
                        stop=True,
                    )
                    if j >= 0:
                        nc.vector.tensor_add(
                            ssc[:, c0 : c0 + P], ssc[:, c0 : c0 + P], maskT_t
                        )
                    p = pt_pool.tile([P, 512], _dt_bf16, tag="pt")
                    nc.scalar.activation(
                        out=p[:, c0:512], in_=ssc[:, c0:512], func=_EXP, scale=SCALE
                    )
                    if j >= 1:
                        nc.gpsimd.memset(p[:, 0:c0], 0.0)
                    slices.append(p)
                    # denominator partials interleaved with the exp stream so
                    # the DVE chain finishes with the chunk, not after it
                    if kb == 2:
                        nc.vector.tensor_add(la, slices[0], slices[2])
                    elif kb == 3:
                        nc.vector.tensor_add(lb, slices[1], slices[3])
                    elif kb >= 4:
                        acc = la if kb % 2 == 0 else lb
                        nc.vector.tensor_add(acc, acc, p)
                    if kb % 4 == 3:
                        drain(1)
                pa = ps_pa.tile([P, 512], _dt_f32, tag="pa")
                for kb in range(nblk):
                    nc.tensor.matmul(
                        pa,
                        lhsT=v_t[:, kb, g2 * P : (g2 + 1) * P],
                        rhs=slices[kb],
                        start=(kb == 0),
                        stop=(kb == nblk - 1),
                    )
                    if kb % 8 == 7:
                        drain(1)
                # combine denominator chains; one all-ones matmul does the
                # partition sum AND the broadcast (out[m,q] = sum_p la[p,q])
                nc.vector.tensor_add(la, la, lb)
                lsum = ps_sw.tile([P, 512], _dt_f32, tag="sw")
                nc.tensor.matmul(lsum, lhsT=ones_t, rhs=la, start=True, stop=True)
                linv = l_pool.tile([P, 512], _dt_f32, tag="linv")
                if qc < 2:
                    # early chunks: DVE is the pacer (little exp work, fast PE
                    # cadence) and its 6-cpe reciprocal backlogs the at-tile
                    # writes; compute linv = exp(-ln(lsum)) on the scalar
                    # engine instead, which has headroom here.
                    nl = l_pool.tile([P, 512], _dt_f32, tag="nl")
                    nc.scalar.activation(
                        out=nl, in_=lsum, func=mybir.ActivationFunctionType.Ln
                    )
                    nc.scalar.activation(out=linv, in_=nl, func=_EXP, scale=-1.0)
                else:
                    nc.vector.reciprocal(linv, lsum)
                nc.vector.tensor_mul(at_t[:, h, :], pa, linv)
            return at_t

        def wo_items(qc, at_t):
            """32 filler closures: wo GEMM for chunk qc, ec-outer/sb-inner so
            each streamed wo piece is reused by 4 s-blocks."""
            items = []
            state = {}
            for ec in range(8):
                for sb4 in range(4):
                    def f(ec=ec, sb4=sb4):
                        if sb4 == 0:
                            wp = wo_pool.tile([P, NH_L, 512], _dt_bf16, tag="wo")
                            nc.scalar.dma_start(
                                out=wp, in_=wor[:, :, ec * 512 : (ec + 1) * 512]
                            )
                            state[ec] = wp
                        wp = state[ec]
                        go = ps_acc.tile([P, 512], _dt_f32, tag="acc")
                        for hb in range(NH_L):
                            nc.tensor.matmul(
                                go,
                                lhsT=at_t[:, hb, sb4 * P : (sb4 + 1) * P],
                                rhs=wp[:, hb, :],
                                start=(hb == 0),
                                stop=(hb == NH_L - 1),
                            )
                        ost = ost_pool.tile([P, 512], _dt_bf16, tag="ost")
                        cp(ost, go)
                        nc.sync.dma_start(
                            out=out_ap[
                                qc * 512 + sb4 * P : qc * 512 + (sb4 + 1) * P,
                                ec * 512 : (ec + 1) * 512,
                            ],
                            in_=ost,
                        )
                    items.append(f)
            return items

        # ================= main schedule =================
        for hi in range(2):
            s0 = hi * 1024
            # rope tables for this half (scalar queue, tiny)
            ctab_t = tabs.tile([P, 1024], _dt_bf16, tag="ct")
            stab_t = tabs.tile([P, 1024], _dt_bf16, tag="st")
            nc.scalar.dma_start(out=ctab_t, in_=cexp_ap[:, s0 : s0 + 1024])
            nc.scalar.dma_start(out=stab_t, in_=sexp_ap[:, s0 : s0 + 1024])
            # V weights + first K piece on scalar queue (3 wkv slots live)
            va = wkv_pool.tile([P, 16, 256], _dt_bf16, tag="wkv")
            nc.scalar.dma_start(out=va, in_=wkvr[:, 0:16, 256:512])
            vb = wkv_pool.tile([P, 16, 256], _dt_bf16, tag="wkv")
            nc.scalar.dma_start(out=vb, in_=wkvr[:, 16:32, 256:512])
            ka = wkv_pool.tile([P, 16, 256], _dt_bf16, tag="wkv")
            nc.scalar.dma_start(out=ka, in_=wkvr[:, 0:16, 0:256])
            # x^T half on sync queue, s-ordered pieces
            xt_t = xt_pool.tile([P, KO, 1024], _dt_bf16, tag="xt")
            # first piece split fine so the very first V chain starts early
            for ci in range(2):
                c0, c1 = ci * 128, (ci + 1) * 128
                nc.sync.dma_start(
                    out=xt_t[:, 0:16, c0:c1], in_=xr[:, 0:16, s0 + c0 : s0 + c1]
                )
                nc.sync.dma_start(
                    out=xt_t[:, 16:32, c0:c1], in_=xr[:, 16:32, s0 + c0 : s0 + c1]
                )
            for pi in range(1, 4):
                nc.sync.dma_start(
                    out=xt_t[:, :, pi * 256 : (pi + 1) * 256],
                    in_=xr[:, :, s0 + pi * 256 : s0 + (pi + 1) * 256],
                )
            qt_t = qt_pool.tile([P, NH_L, 1024], _dt_bf16, tag="qt")

            # ---- projections: V sb0-7, K sc0+sc1, Q (all heads) ----
            for sb_l in range(8):
                v_group(xt_t, hi, sb_l, va, vb)
            kb_ = wkv_pool.tile([P, 16, 256], _dt_bf16, tag="wkv")
            nc.scalar.dma_start(out=kb_, in_=wkvr[:, 16:32, 0:256])
            for sc in range(2):
                for g2 in range(NKV_L):
                    k_group(xt_t, hi, sc, g2, ka, kb_, ctab_t, stab_t)
            for hp in range(4):
                wa = wq_pool.tile([P, 16, 256], _dt_bf16, tag="wq")
                nc.sync.dma_start(
                    out=wa, in_=wqr[:, 0:16, hp * 256 : (hp + 1) * 256]
                )
                wb = wq_pool.tile([P, 16, 256], _dt_bf16, tag="wq")
                nc.sync.dma_start(
                    out=wb, in_=wqr[:, 16:32, hp * 256 : (hp + 1) * 256]
                )
                # last group reads the 'wa' piece backwards so it frees early
                q_group(xt_t, qt_t, hp, 0, 0, wa, wb, ctab_t, stab_t)
                q_group(xt_t, qt_t, hp, 1, 0, wa, wb, ctab_t, stab_t)
                q_group(xt_t, qt_t, hp, 0, 1, wa, wb, ctab_t, stab_t)
                q_group(xt_t, qt_t, hp, 1, 1, wa, wb, ctab_t, stab_t, rev=True)

            # ---- attention qc_a = 2*hi, then qc_b = 2*hi+1 ----
            qc_a = 2 * hi
            at_a = attention(qc_a, qt_t)
            drain_all()
            fillers.extend(wo_items(qc_a, at_a))
            qc_b = 2 * hi + 1
            at_b = attention(qc_b, qt_t)
            # flush wo(qc_a) leftovers now: attention(qc_a+2) will reuse
            # at(qc_a)'s slot (at_pool bufs=2), so its readers must all be
            # emitted before then.
            drain_all()
            if hi == 0:
                # wo(qc1) items become the fillers for half-1 attn(qc2)
                fillers.extend(wo_items(qc_b, at_b))
            else:
                for f in wo_items(qc_b, at_b):
                    f()


def _split_excess_waits(nc, limit=1):
    """Walrus (pinned build) rejects instructions carrying more than ~2
    semaphore waits.  Hoist excess waits onto same-engine no-ops inserted
    immediately before the offending instruction."""
    ctr = [0]
    for bb in nc.main_func.blocks:
        insts = list(bb.instructions)
        out = []
        changed = False
        for ins in insts:
            si = ins.sync_info
            waits = list(si.on_wait) if si and si.on_wait else []
            if len(waits) > limit:
                keep = waits[:limit]
                rest = waits[limit:]
                for i in range(0, len(rest), limit):
                    nop = mybir.InstNoOp(name=f"I-waitsplit-{ctr[0]}", ins=[], outs=[])
                    ctr[0] += 1
                    nop.engine = ins.engine
                    nop.sync_info = mybir.SyncInfo(
                        on_wait=rest[i : i + limit], on_update=[]
                    )
                    nc.register_instruction(nop)
                    out.append(nop)
                si.on_wait = keep
                changed = True
            out.append(ins)
        if changed:
            bb.instructions = out
    return ctr[0]


_PROGRAM_CACHE = {}


def build_program():
    if "nc" in _PROGRAM_CACHE:
        return _PROGRAM_CACHE["nc"]
    nc = bass.Bass("TRN2", target_bir_lowering=False, debug=False, num_devices=N_CORES)
    aps = {
        "xT": nc.dram_tensor("xT", [D, S], _dt_bf16, kind="ExternalInput").ap(),
        "wqT": nc.dram_tensor("wqT", [D, NH_L * DH], _dt_bf16, kind="ExternalInput").ap(),
        "wkvT": nc.dram_tensor("wkvT", [D, 512], _dt_bf16, kind="ExternalInput").ap(),
        "woT": nc.dram_tensor("woT", [NH_L * DH, D], _dt_bf16, kind="ExternalInput").ap(),
        "cexp": nc.dram_tensor("cexp", [P, S], _dt_bf16, kind="ExternalInput").ap(),
        "sexp": nc.dram_tensor("sexp", [P, S], _dt_bf16, kind="ExternalInput").ap(),
        "perm": nc.dram_tensor("perm", [P, P], _dt_bf16, kind="ExternalInput").ap(),
        "maskT": nc.dram_tensor("maskT", [P, P], _dt_f32, kind="ExternalInput").ap(),
        "ones": nc.dram_tensor("ones", [P, P], _dt_bf16, kind="ExternalInput").ap(),
        "out": nc.dram_tensor("out", [S, D], _dt_bf16, kind="ExternalOutput").ap(),
    }
    with tile.TileContext(nc) as tc:
        _emit(tc, aps)
    _split_excess_waits(nc, limit=1)
    _PROGRAM_CACHE["nc"] = nc
    return nc


def make_in_maps(x, freqs_cos, freqs_sin, mask, wq, wk, wv, wo):
    x = np.asarray(x, np.float32)
    freqs_cos = np.asarray(freqs_cos, np.float32)
    freqs_sin = np.asarray(freqs_sin, np.float32)
    mask = np.asarray(mask, np.float32)
    wq = np.asarray(wq, np.float32)
    wk = np.asarray(wk, np.float32)
    wv = np.asarray(wv, np.float32)
    wo = np.asarray(wo, np.float32)

    xb = [np.ascontiguousarray(x[b].T).astype(BF16) for b in range(2)]  # (4096, 2048)
    cexp = np.repeat(freqs_cos.T, 2, axis=0).astype(BF16)  # (128, 2048)
    sx = np.repeat(freqs_sin.T, 2, axis=0).astype(np.float32)
    sx[0::2] *= -1.0
    sexp = sx.astype(BF16)
    perm = np.zeros((P, P), np.float32)
    idx = np.arange(P)
    perm[idx, idx ^ 1] = 1.0
    perm = perm.astype(BF16)
    maskT = np.ascontiguousarray(mask[:P, :P].T, dtype=np.float32)
    ones = np.ones((P, P), dtype=BF16)

    in_maps = []
    for core in range(N_CORES):
        b, tp = core // 4, core % 4
        wqT = np.ascontiguousarray(wq[tp * 1024 : (tp + 1) * 1024].T).astype(BF16)
        wkT = wk[tp * 256 : (tp + 1) * 256].T.astype(BF16)  # (4096, 256)
        wvT = wv[tp * 256 : (tp + 1) * 256].T.astype(BF16)
        wkvT = np.ascontiguousarray(np.concatenate([wkT, wvT], axis=1))
        woT = np.ascontiguousarray(wo[:, tp * 1024 : (tp + 1) * 1024].T).astype(BF16)
        in_maps.append(
            {
                "xT": xb[b],
                "wqT": wqT,
                "wkvT": wkvT,
                "woT": woT,
                "cexp": cexp,
                "sexp": sexp,
                "perm": perm,
                "maskT": maskT,
                "ones": ones,
            }
        )
    return in_maps


def run(inputs, trace=False):
    nc = build_program()
    in_maps = make_in_maps(
        inputs["x"],
        inputs["freqs_cos"],
        inputs["freqs_sin"],
        inputs["mask"],
        inputs["wq"],
        inputs["wk"],
        inputs["wv"],
        inputs["wo"],
    )
    res = run_bass_kernel_spmd(nc, in_maps, list(range(N_CORES)), trace=trace)
    out = np.zeros((2, S, D), np.float32)
    for core in range(N_CORES):
        out[core // 4] += np.asarray(res.results[core]["out"]).astype(np.float32)
    return out, res


def kernel(x, freqs_cos, freqs_sin, positions, mask, wq, wk, wv, wo):
    out, _ = run(
        {
            "x": x,
            "freqs_cos": freqs_cos,
            "freqs_sin": freqs_sin,
            "mask": mask,
            "wq": wq,
            "wk": wk,
            "wv": wv,
            "wo": wo,
        }
    )
    return out


# revision 30
# speedup vs baseline: 1.0159x; 1.0159x over previous
"""Trainium2 Bass kernel for Mistral-style MHA prefill (sparse_attention).

Problem: B=2, S=2048, DIM=4096, 32 q heads / 8 kv heads, head_dim=128,
sliding window 2048 (== S, pure causal), RoPE, fp32 reference.

Sharding (8 cores): data-parallel over batch (2) x tensor-parallel over heads
(4).  Core c = b*4 + tp handles batch b, q-heads [tp*8, tp*8+8), kv-heads
[tp*2, tp*2+2).  wq/wk/wv sharded column-wise, wo row-wise; the all-reduce
after wo is done on the host (f32 sum of the 4 bf16 partials per batch).

v2 dataflow ("all-transposed attention", zero PE transposes):
  - Q^T/K^T produced as [dh, s] (lhsT=w chunk, rhs=x^T chunk); V natural
    [s, dh] (lhsT=x^T chunk, rhs=wv).  RoPE via pair-swap permutation matmul
    + 3 DVE ops (as in v1).
  - scores computed TRANSPOSED: S^T[k, q-chunk512] = matmul(lhsT=K^T 128-col
    block, rhs=Q^T chunk).  Causal: k-blocks <= q-block only; the 4 diagonal
    blocks are column-restricted with a transposed 128x128 triangular mask.
  - P^T = exp(scale * S^T) on the scalar engine into a ring of [128,512]
    bf16 slices; invalid prefixes of diagonal slices zeroed by gpsimd memset.
  - PV transposed: A^T[dh, q] += matmul(lhsT=V[kb-block, dh], rhs=P^T slice)
    -- V natural is the stationary operand, so NO P/A transposes at all and
    A^T is directly the lhsT for the wo GEMM.
  - softmax row sums (= partition-dim sums of P^T): DVE accumulates slices
    into two bf16 chains, one PSUM-accumulated all-ones matmul does the
    partition sum AND the broadcast, DVE reciprocal, and the normalization
    is fused into the PSUM->SBUF copy of A^T (tensor_mul by linv).
  - wo: out[s-block, e] = sum_hb A^T[hb].T @ woT[hb] with woT streamed per
    512-e-column piece; output partials written bf16.
  - Scheduling: per half -> per 512-q-chunk.  exp on scalar is ~1.6x slower
    than the attention matmuls, so wo(qc-1) groups and next-phase projection
    groups are explicitly interleaved into the attention emission as PE
    filler work.  Two DMA queues (sync: x/wq/out, scalar: tables/wkv/wo)
    with weights-first ordering to minimize the startup stall.
"""

import os
import sys

import numpy as np

for _p in ("/opt/trn_rl_repo",):
    if _p not in sys.path and os.path.isdir(_p):
        sys.path.insert(0, _p)

import ml_dtypes  # noqa: E402

import concourse.bass as bass  # noqa: E402
import concourse.bass_isa as bass_isa  # noqa: E402
import concourse.mybir as mybir  # noqa: E402
import concourse.tile as tile  # noqa: E402
from concourse.bass_utils import run_bass_kernel_spmd  # noqa: E402

BF16 = ml_dtypes.bfloat16


def _install_drain_split_patch():
    """The pinned walrus rejects Tile's kernel-tail Drain when it carries more
    than ~2 semaphore waits ("Too many sync wait commands").  Split the global
    drain's waits across trailing sync-engine nops (1 wait each)."""
    if getattr(tile.TileContext, "_drain_split_patched", False):
        return
    from concourse.vector_clock import ScopedClock

    limit = 1

    def _patched_dab(self, tick_clock, wait_clock):
        drain_inst = self.nc.sync.drain()
        raw = drain_inst.ins
        wait_clock.add_sem_waits(raw, ScopedClock({None: tick_clock.global_clock}))
        si = raw.sync_info
        waits = list(si.on_wait or [])
        if len(waits) > limit:
            si.on_wait = waits[:limit]
            for i in range(limit, len(waits), limit):
                nraw = self.nc.sync.nop().ins
                nsi = nraw.sync_info
                if nsi is None:
                    nraw.sync_info = mybir.SyncInfo(
                        on_wait=waits[i : i + limit], on_update=[]
                    )
                else:
                    nsi.on_wait = list(nsi.on_wait or []) + waits[i : i + limit]
        self.nc.all_engine_barrier()
        popped = self.nc._tile_sem_poison_stack.pop()
        assert popped is self._sem_poison
        self.nc.clear_and_free_semaphores(list(self.sems.allocated().values()))
        self.nc.all_engine_barrier()

    tile.TileContext._drain_and_barrier = _patched_dab
    tile.TileContext._drain_split_patched = True


_install_drain_split_patch()

P = 128
S = 2048
D = 4096
KO = D // P  # 32 contraction chunks
NH_L = 8  # q heads per core
NKV_L = 2  # kv heads per core
DH = 128
SCALE = float(DH) ** -0.5
N_CORES = 8

_dt_f32 = mybir.dt.float32
_dt_bf16 = mybir.dt.bfloat16
_EXP = mybir.ActivationFunctionType.Exp


def _emit(tc, aps):
    nc = tc.nc
    xr = aps["xT"].rearrange("(ko p) s -> p ko s", p=P)  # [128, 32, 2048]
    wqr = aps["wqT"].rearrange("(ko p) o -> p ko o", p=P)  # [128, 32, 1024]
    wkvr = aps["wkvT"].rearrange("(ko p) o -> p ko o", p=P)  # [128, 32, 512]
    wor = aps["woT"].rearrange("(ho p) e -> p ho e", p=P)  # [128, 8, 4096]
    out_ap = aps["out"]  # [2048, 4096] bf16
    cexp_ap = aps["cexp"]  # [128, 2048] bf16
    sexp_ap = aps["sexp"]

    from contextlib import ExitStack

    with ExitStack() as g:
        consts = g.enter_context(tc.tile_pool(name="consts", bufs=1))
        tabs = g.enter_context(tc.tile_pool(name="tabs", bufs=1))
        kt_pool = g.enter_context(tc.tile_pool(name="ktp", bufs=1))
        vt_pool = g.enter_context(tc.tile_pool(name="vtp", bufs=1))
        xt_pool = g.enter_context(tc.tile_pool(name="xtp", bufs=1))
        qt_pool = g.enter_context(tc.tile_pool(name="qtp", bufs=1))
        wq_pool = g.enter_context(tc.tile_pool(name="wqp", bufs=3))
        wkv_pool = g.enter_context(tc.tile_pool(name="wkvp", bufs=3))
        pt_pool = g.enter_context(tc.tile_pool(name="ptp", bufs=16))
        at_pool = g.enter_context(tc.tile_pool(name="atp", bufs=2))
        wo_pool = g.enter_context(tc.tile_pool(name="wop", bufs=2))
        l_pool = g.enter_context(tc.tile_pool(name="lp", bufs=1))
        ost_pool = g.enter_context(tc.tile_pool(name="ostp", bufs=2))
        tmp_pool = g.enter_context(tc.tile_pool(name="tmpp", bufs=2))
        # PSUM: 3 + 1 + 2 + 2 = 8 banks
        ps_sc = g.enter_context(tc.tile_pool(name="ps_sc", bufs=3, space="PSUM"))
        ps_sw = g.enter_context(tc.tile_pool(name="ps_sw", bufs=1, space="PSUM"))
        ps_acc = g.enter_context(tc.tile_pool(name="ps_acc", bufs=2, space="PSUM"))
        ps_pa = g.enter_context(tc.tile_pool(name="ps_pa", bufs=2, space="PSUM"))

        perm_t = consts.tile([P, P], _dt_bf16)
        maskT_t = consts.tile([P, P], _dt_f32)
        ones_t = consts.tile([P, P], _dt_bf16)
        nc.scalar.dma_start(out=perm_t, in_=aps["perm"])
        nc.scalar.dma_start(out=maskT_t, in_=aps["maskT"])
        nc.scalar.dma_start(out=ones_t, in_=aps["ones"])

        for wi in range(6):
            wup = ps_sc.tile([P, 512], _dt_f32, tag="sc")
            nc.tensor.matmul(
                wup[:, :P], lhsT=perm_t, rhs=perm_t, start=True, stop=True
            )

        kt_t = kt_pool.tile([P, NKV_L, S], _dt_bf16)  # K^T roped, full S
        v_t = vt_pool.tile([P, 16, NKV_L * DH], _dt_bf16)  # V natural, full S

        cp_flip = [0]

        def cp(out, in_):
            if cp_flip[0] % 2 == 0:
                nc.scalar.copy(out=out, in_=in_)
            else:
                nc.vector.tensor_copy(out=out, in_=in_)
            cp_flip[0] += 1

        # ---- filler plumbing: closures emitting ~1 PE accumulation group ----
        fillers = []

        def drain(n):
            k = min(n, len(fillers))
            for _ in range(k):
                fillers.pop(0)()

        def drain_all():
            drain(len(fillers))

        def rope(dst, gsrc, ctab_t, stab_t, sc):
            """dst (sbuf bf16 [128,512]) <- rope(gsrc (psum f32 [128,512]))."""
            nc.scalar.copy(out=dst, in_=gsrc)
            sw = ps_sw.tile([P, 512], _dt_f32, tag="sw")
            nc.tensor.matmul(sw, lhsT=perm_t, rhs=dst, start=True, stop=True)
            t1 = tmp_pool.tile([P, 512], _dt_bf16, tag="t1")
            nc.vector.tensor_mul(t1, sw, stab_t[:, sc * 512 : (sc + 1) * 512])
            nc.vector.tensor_mul(dst, dst, ctab_t[:, sc * 512 : (sc + 1) * 512])
            nc.vector.tensor_add(dst, dst, t1)

        def v_group(xt_t, hi, sb_l, va, vb):
            sb = hi * 8 + sb_l
            gv = ps_acc.tile([P, 512], _dt_f32, tag="acc")
            for ko in range(KO):
                w = va if ko < 16 else vb
                nc.tensor.matmul(
                    gv[:, :256],
                    lhsT=xt_t[:, ko, sb_l * P : (sb_l + 1) * P],
                    rhs=w[:, ko % 16, :],
                    start=(ko == 0),
                    stop=(ko == KO - 1),
                )
            cp(v_t[:, sb, :], gv[:, :256])

        def k_group(xt_t, hi, sc, g2, ka, kb_, ctab_t, stab_t):
            gk = ps_acc.tile([P, 512], _dt_f32, tag="acc")
            for ko in range(KO):
                w = ka if ko < 16 else kb_
                nc.tensor.matmul(
                    gk,
                    lhsT=w[:, ko % 16, g2 * P : (g2 + 1) * P],
                    rhs=xt_t[:, ko, sc * 512 : (sc + 1) * 512],
                    start=(ko == 0),
                    stop=(ko == KO - 1),
                )
            rope(
                kt_t[:, g2, hi * 1024 + sc * 512 : hi * 1024 + (sc + 1) * 512],
                gk, ctab_t, stab_t, sc,
            )

        def q_group(xt_t, qt_t, hp, h2, sc, wa, wb, ctab_t, stab_t, rev=False):
            gq = ps_acc.tile([P, 512], _dt_f32, tag="acc")
            kos = range(KO - 1, -1, -1) if rev else range(KO)
            first = KO - 1 if rev else 0
            last = 0 if rev else KO - 1
            for ko in kos:
                w = wa if ko < 16 else wb
                nc.tensor.matmul(
                    gq,
                    lhsT=w[:, ko % 16, h2 * P : (h2 + 1) * P],
                    rhs=xt_t[:, ko, sc * 512 : (sc + 1) * 512],
                    start=(ko == first),
                    stop=(ko == last),
                )
            rope(
                qt_t[:, hp * 2 + h2, sc * 512 : (sc + 1) * 512],
                gq, ctab_t, stab_t, sc,
            )

        def attention(qc, qt_t):
            """S^T attention for 512-q chunk qc (global 0..3), all 8 heads.
            Fillers drained inside the kb loops to cover the exp deficit."""
            qc_l = qc & 1
            at_t = at_pool.tile([P, NH_L, 512], _dt_bf16, tag="at")
            nblk = 4 * qc + 4
            for h in range(NH_L):
                g2 = h // 4
                la = l_pool.tile([P, 512], _dt_bf16, tag="lacc_a")
                lb = l_pool.tile([P, 512], _dt_bf16, tag="lacc_b")
                slices = []
                for kb in range(nblk):
                    j = kb - 4 * qc
                    c0 = j * P if j > 0 else 0
                    ssc = ps_sc.tile([P, 512], _dt_f32, tag="sc")
                    nc.tensor.matmul(
                        ssc[:, c0:512],
                        lhsT=kt_t[:, g2, kb * P : (kb + 1) * P],
                        rhs=qt_t[:, h, qc_l * 512 + c0 : qc_l * 512 + 512],
                        start=True,
                        stop=True,
                    )
                    if j >= 0:
                        nc.vector.tensor_add(
                            ssc[:, c0 : c0 + P], ssc[:, c0 : c0 + P], maskT_t
                        )
                    p = pt_pool.tile([P, 512], _dt_bf16, tag="pt")
                    nc.scalar.activation(
                        out=p[:, c0:512], in_=ssc[:, c0:512], func=_EXP, scale=SCALE
                    )
                    if j >= 1:
                        nc.gpsimd.memset(p[:, 0:c0], 0.0)
                    slices.append(p)
                    # denominator partials interleaved with the exp stream so
                    # the DVE chain finishes with the chunk, not after it
                    if kb == 2:
                        nc.vector.tensor_add(la, slices[0], slices[2])
                    elif kb == 3:
                        nc.vector.tensor_add(lb, slices[1], slices[3])
                    elif kb >= 4:
                        acc = la if kb % 2 == 0 else lb
                        nc.vector.tensor_add(acc, acc, p)
                    if kb % 4 == 3:
                        drain(1)
                pa = ps_pa.tile([P, 512], _dt_f32, tag="pa")
                for kb in range(nblk):
                    nc.tensor.matmul(
                        pa,
                        lhsT=v_t[:, kb, g2 * P : (g2 + 1) * P],
                        rhs=slices[kb],
                        start=(kb == 0),
                        stop=(kb == nblk - 1),
                    )
                    if kb % 8 == 7:
                        drain(1)
                # combine denominator chains; one all-ones matmul does the
                # partition sum AND the broadcast (out[m,q] = sum_p la[p,q])
                nc.vector.tensor_add(la, la, lb)
                lsum = ps_sw.tile([P, 512], _dt_f32, tag="sw")
                nc.tensor.matmul(lsum, lhsT=ones_t, rhs=la, start=True, stop=True)
                linv = l_pool.tile([P, 512], _dt_f32, tag="linv")
                if qc < 2 or h == 7:
                    # early chunks: DVE is the pacer (little exp work, fast PE
                    # cadence) and its 6-cpe reciprocal backlogs the at-tile
                    # writes; compute linv = exp(-ln(lsum)) on the scalar
                    # engine instead, which has headroom here.
                    nl = l_pool.tile([P, 512], _dt_f32, tag="nl")
                    nc.scalar.activation(
                        out=nl, in_=lsum, func=mybir.ActivationFunctionType.Ln
                    )
                    nc.scalar.activation(out=linv, in_=nl, func=_EXP, scale=-1.0)
                else:
                    nc.vector.reciprocal(linv, lsum)
                nc.vector.tensor_mul(at_t[:, h, :], pa, linv)
            return at_t

        def wo_items(qc, at_t):
            """32 filler closures: wo GEMM for chunk qc, ec-outer/sb-inner so
            each streamed wo piece is reused by 4 s-blocks."""
            items = []
            state = {}
            for ec in range(8):
                for sb4 in range(4):
                    def f(ec=ec, sb4=sb4):
                        if sb4 == 0:
                            wp = wo_pool.tile([P, NH_L, 512], _dt_bf16, tag="wo")
                            nc.scalar.dma_start(
                                out=wp, in_=wor[:, :, ec * 512 : (ec + 1) * 512]
                            )
                            state[ec] = wp
                        wp = state[ec]
                        go = ps_acc.tile([P, 512], _dt_f32, tag="acc")
                        for hb in range(NH_L):
                            nc.tensor.matmul(
                                go,
                                lhsT=at_t[:, hb, sb4 * P : (sb4 + 1) * P],
                                rhs=wp[:, hb, :],
                                start=(hb == 0),
                                stop=(hb == NH_L - 1),
                            )
                        ost = ost_pool.tile([P, 512], _dt_bf16, tag="ost")
                        cp(ost, go)
                        nc.sync.dma_start(
                            out=out_ap[
                                qc * 512 + sb4 * P : qc * 512 + (sb4 + 1) * P,
                                ec * 512 : (ec + 1) * 512,
                            ],
                            in_=ost,
                        )
                    items.append(f)
            return items

        # ================= main schedule =================
        for hi in range(2):
            s0 = hi * 1024
            # rope tables for this half (scalar queue, tiny)
            ctab_t = tabs.tile([P, 1024], _dt_bf16, tag="ct")
            stab_t = tabs.tile([P, 1024], _dt_bf16, tag="st")
            nc.scalar.dma_start(out=ctab_t, in_=cexp_ap[:, s0 : s0 + 1024])
            nc.scalar.dma_start(out=stab_t, in_=sexp_ap[:, s0 : s0 + 1024])
            # V weights + first K piece on scalar queue (3 wkv slots live)
            va = wkv_pool.tile([P, 16, 256], _dt_bf16, tag="wkv")
            nc.scalar.dma_start(out=va, in_=wkvr[:, 0:16, 256:512])
            vb = wkv_pool.tile([P, 16, 256], _dt_bf16, tag="wkv")
            nc.scalar.dma_start(out=vb, in_=wkvr[:, 16:32, 256:512])
            ka = wkv_pool.tile([P, 16, 256], _dt_bf16, tag="wkv")
            nc.scalar.dma_start(out=ka, in_=wkvr[:, 0:16, 0:256])
            # x^T half on sync queue, s-ordered pieces
            xt_t = xt_pool.tile([P, KO, 1024], _dt_bf16, tag="xt")
            # first piece split fine so the very first V chain starts early
            for ci in range(2):
                c0, c1 = ci * 128, (ci + 1) * 128
                nc.sync.dma_start(
                    out=xt_t[:, 0:16, c0:c1], in_=xr[:, 0:16, s0 + c0 : s0 + c1]
                )
                nc.sync.dma_start(
                    out=xt_t[:, 16:32, c0:c1], in_=xr[:, 16:32, s0 + c0 : s0 + c1]
                )
            for pi in range(1, 4):
                nc.sync.dma_start(
                    out=xt_t[:, :, pi * 256 : (pi + 1) * 256],
                    in_=xr[:, :, s0 + pi * 256 : s0 + (pi + 1) * 256],
                )
            qt_t = qt_pool.tile([P, NH_L, 1024], _dt_bf16, tag="qt")

            # ---- projections: V sb0-7, K sc0+sc1, Q (all heads) ----
            for sb_l in range(8):
                v_group(xt_t, hi, sb_l, va, vb)
            kb_ = wkv_pool.tile([P, 16, 256], _dt_bf16, tag="wkv")
            nc.scalar.dma_start(out=kb_, in_=wkvr[:, 16:32, 0:256])
            for sc in range(2):
                for g2 in range(NKV_L):
                    k_group(xt_t, hi, sc, g2, ka, kb_, ctab_t, stab_t)
            for hp in range(4):
                wa = wq_pool.tile([P, 16, 256], _dt_bf16, tag="wq")
                nc.sync.dma_start(
                    out=wa, in_=wqr[:, 0:16, hp * 256 : (hp + 1) * 256]
                )
                wb = wq_pool.tile([P, 16, 256], _dt_bf16, tag="wq")
                nc.sync.dma_start(
                    out=wb, in_=wqr[:, 16:32, hp * 256 : (hp + 1) * 256]
                )
                # last group reads the 'wa' piece backwards so it frees early
                q_group(xt_t, qt_t, hp, 0, 0, wa, wb, ctab_t, stab_t)
                q_group(xt_t, qt_t, hp, 1, 0, wa, wb, ctab_t, stab_t)
                q_group(xt_t, qt_t, hp, 0, 1, wa, wb, ctab_t, stab_t)
                q_group(xt_t, qt_t, hp, 1, 1, wa, wb, ctab_t, stab_t, rev=True)

            # ---- attention qc_a = 2*hi, then qc_b = 2*hi+1 ----
            qc_a = 2 * hi
            at_a = attention(qc_a, qt_t)
            drain_all()
            fillers.extend(wo_items(qc_a, at_a))
            qc_b = 2 * hi + 1
            at_b = attention(qc_b, qt_t)
            # flush wo(qc_a) leftovers now: attention(qc_a+2) will reuse
            # at(qc_a)'s slot (at_pool bufs=2), so its readers must all be
            # emitted before then.
            drain_all()
            if hi == 0:
                # wo(qc1) items become the fillers for half-1 attn(qc2)
                fillers.extend(wo_items(qc_b, at_b))
            else:
                for f in wo_items(qc_b, at_b):
                    f()


def _split_excess_waits(nc, limit=1):
    """Walrus (pinned build) rejects instructions carrying more than ~2
    semaphore waits.  Hoist excess waits onto same-engine no-ops inserted
    immediately before the offending instruction."""
    ctr = [0]
    for bb in nc.main_func.blocks:
        insts = list(bb.instructions)
        out = []
        changed = False
        for ins in insts:
            si = ins.sync_info
            waits = list(si.on_wait) if si and si.on_wait else []
            if len(waits) > limit:
                keep = waits[:limit]
                rest = waits[limit:]
                for i in range(0, len(rest), limit):
                    nop = mybir.InstNoOp(name=f"I-waitsplit-{ctr[0]}", ins=[], outs=[])
                    ctr[0] += 1
                    nop.engine = ins.engine
                    nop.sync_info = mybir.SyncInfo(
                        on_wait=rest[i : i + limit], on_update=[]
                    )
                    nc.register_instruction(nop)
                    out.append(nop)
                si.on_wait = keep
                changed = True
            out.append(ins)
        if changed:
            bb.instructions = out
    return ctr[0]


_PROGRAM_CACHE = {}


def build_program():
    if "nc" in _PROGRAM_CACHE:
        return _PROGRAM_CACHE["nc"]
    nc = bass.Bass("TRN2", target_bir_lowering=False, debug=False, num_devices=N_CORES)
    aps = {
        "xT": nc.dram_tensor("xT", [D, S], _dt_bf16, kind="ExternalInput").ap(),
        "wqT": nc.dram_tensor("wqT", [D, NH_L * DH], _dt_bf16, kind="ExternalInput").ap(),
        "wkvT": nc.dram_tensor("wkvT", [D, 512], _dt_bf16, kind="ExternalInput").ap(),
        "woT": nc.dram_tensor("woT", [NH_L * DH, D], _dt_bf16, kind="ExternalInput").ap(),
        "cexp": nc.dram_tensor("cexp", [P, S], _dt_bf16, kind="ExternalInput").ap(),
        "sexp": nc.dram_tensor("sexp", [P, S], _dt_bf16, kind="ExternalInput").ap(),
        "perm": nc.dram_tensor("perm", [P, P], _dt_bf16, kind="ExternalInput").ap(),
        "maskT": nc.dram_tensor("maskT", [P, P], _dt_f32, kind="ExternalInput").ap(),
        "ones": nc.dram_tensor("ones", [P, P], _dt_bf16, kind="ExternalInput").ap(),
        "out": nc.dram_tensor("out", [S, D], _dt_bf16, kind="ExternalOutput").ap(),
    }
    with tile.TileContext(nc) as tc:
        _emit(tc, aps)
    _split_excess_waits(nc, limit=1)
    _PROGRAM_CACHE["nc"] = nc
    return nc


def make_in_maps(x, freqs_cos, freqs_sin, mask, wq, wk, wv, wo):
    x = np.asarray(x, np.float32)
    freqs_cos = np.asarray(freqs_cos, np.float32)
    freqs_sin = np.asarray(freqs_sin, np.float32)
    mask = np.asarray(mask, np.float32)
    wq = np.asarray(wq, np.float32)
    wk = np.asarray(wk, np.float32)
    wv = np.asarray(wv, np.float32)
    wo = np.asarray(wo, np.float32)

    xb = [np.ascontiguousarray(x[b].T).astype(BF16) for b in range(2)]  # (4096, 2048)
    cexp = np.repeat(freqs_cos.T, 2, axis=0).astype(BF16)  # (128, 2048)
    sx = np.repeat(freqs_sin.T, 2, axis=0).astype(np.float32)
    sx[0::2] *= -1.0
    sexp = sx.astype(BF16)
    perm = np.zeros((P, P), np.float32)
    idx = np.arange(P)
    perm[idx, idx ^ 1] = 1.0
    perm = perm.astype(BF16)
    maskT = np.ascontiguousarray(mask[:P, :P].T, dtype=np.float32)
    ones = np.ones((P, P), dtype=BF16)

    in_maps = []
    for core in range(N_CORES):
        b, tp = core // 4, core % 4
        wqT = np.ascontiguousarray(wq[tp * 1024 : (tp + 1) * 1024].T).astype(BF16)
        wkT = wk[tp * 256 : (tp + 1) * 256].T.astype(BF16)  # (4096, 256)
        wvT = wv[tp * 256 : (tp + 1) * 256].T.astype(BF16)
        wkvT = np.ascontiguousarray(np.concatenate([wkT, wvT], axis=1))
        woT = np.ascontiguousarray(wo[:, tp * 1024 : (tp + 1) * 1024].T).astype(BF16)
        in_maps.append(
            {
                "xT": xb[b],
                "wqT": wqT,
                "wkvT": wkvT,
                "woT": woT,
                "cexp": cexp,
                "sexp": sexp,
                "perm": perm,
                "maskT": maskT,
                "ones": ones,
            }
        )
    return in_maps


def run(inputs, trace=False):
    nc = build_program()
    in_maps = make_in_maps(
        inputs["x"],
        inputs["freqs_cos"],
        inputs["freqs_sin"],
        inputs["mask"],
        inputs["wq"],
        inputs["wk"],
        inputs["wv"],
        inputs["wo"],
    )
    res = run_bass_kernel_spmd(nc, in_maps, list(range(N_CORES)), trace=trace)
    out = np.zeros((2, S, D), np.float32)
    for core in range(N_CORES):
        out[core // 4] += np.asarray(res.results[core]["out"]).astype(np.float32)
    return out, res


def kernel(x, freqs_cos, freqs_sin, positions, mask, wq, wk, wv, wo):
    out, _ = run(
        {
            "x": x,
            "freqs_cos": freqs_cos,
            "freqs_sin": freqs_sin,
            "mask": mask,
            "wq": wq,
            "wk": wk,
            "wv": wv,
            "wo": wo,
        }
    )
    return out


# revision 32
# speedup vs baseline: 1.0282x; 1.0121x over previous
"""Trainium2 Bass kernel for Mistral-style MHA prefill (sparse_attention).

Problem: B=2, S=2048, DIM=4096, 32 q heads / 8 kv heads, head_dim=128,
sliding window 2048 (== S, pure causal), RoPE, fp32 reference.

Sharding (8 cores): data-parallel over batch (2) x tensor-parallel over heads
(4).  Core c = b*4 + tp handles batch b, q-heads [tp*8, tp*8+8), kv-heads
[tp*2, tp*2+2).  wq/wk/wv sharded column-wise, wo row-wise; the all-reduce
after wo is done on the host (f32 sum of the 4 bf16 partials per batch).

v2 dataflow ("all-transposed attention", zero PE transposes):
  - Q^T/K^T produced as [dh, s] (lhsT=w chunk, rhs=x^T chunk); V natural
    [s, dh] (lhsT=x^T chunk, rhs=wv).  RoPE via pair-swap permutation matmul
    + 3 DVE ops (as in v1).
  - scores computed TRANSPOSED: S^T[k, q-chunk512] = matmul(lhsT=K^T 128-col
    block, rhs=Q^T chunk).  Causal: k-blocks <= q-block only; the 4 diagonal
    blocks are column-restricted with a transposed 128x128 triangular mask.
  - P^T = exp(scale * S^T) on the scalar engine into a ring of [128,512]
    bf16 slices; invalid prefixes of diagonal slices zeroed by gpsimd memset.
  - PV transposed: A^T[dh, q] += matmul(lhsT=V[kb-block, dh], rhs=P^T slice)
    -- V natural is the stationary operand, so NO P/A transposes at all and
    A^T is directly the lhsT for the wo GEMM.
  - softmax row sums (= partition-dim sums of P^T): DVE accumulates slices
    into two bf16 chains, one PSUM-accumulated all-ones matmul does the
    partition sum AND the broadcast, DVE reciprocal, and the normalization
    is fused into the PSUM->SBUF copy of A^T (tensor_mul by linv).
  - wo: out[s-block, e] = sum_hb A^T[hb].T @ woT[hb] with woT streamed per
    512-e-column piece; output partials written bf16.
  - Scheduling: per half -> per 512-q-chunk.  exp on scalar is ~1.6x slower
    than the attention matmuls, so wo(qc-1) groups and next-phase projection
    groups are explicitly interleaved into the attention emission as PE
    filler work.  Two DMA queues (sync: x/wq/out, scalar: tables/wkv/wo)
    with weights-first ordering to minimize the startup stall.
"""

import os
import sys

import numpy as np

for _p in ("/opt/trn_rl_repo",):
    if _p not in sys.path and os.path.isdir(_p):
        sys.path.insert(0, _p)

import ml_dtypes  # noqa: E402

import concourse.bass as bass  # noqa: E402
import concourse.bass_isa as bass_isa  # noqa: E402
import concourse.mybir as mybir  # noqa: E402
import concourse.tile as tile  # noqa: E402
from concourse.bass_utils import run_bass_kernel_spmd  # noqa: E402

BF16 = ml_dtypes.bfloat16


def _install_drain_split_patch():
    """The pinned walrus rejects Tile's kernel-tail Drain when it carries more
    than ~2 semaphore waits ("Too many sync wait commands").  Split the global
    drain's waits across trailing sync-engine nops (1 wait each)."""
    if getattr(tile.TileContext, "_drain_split_patched", False):
        return
    from concourse.vector_clock import ScopedClock

    limit = 1

    def _patched_dab(self, tick_clock, wait_clock):
        drain_inst = self.nc.sync.drain()
        raw = drain_inst.ins
        wait_clock.add_sem_waits(raw, ScopedClock({None: tick_clock.global_clock}))
        si = raw.sync_info
        waits = list(si.on_wait or [])
        if len(waits) > limit:
            si.on_wait = waits[:limit]
            for i in range(limit, len(waits), limit):
                nraw = self.nc.sync.nop().ins
                nsi = nraw.sync_info
                if nsi is None:
                    nraw.sync_info = mybir.SyncInfo(
                        on_wait=waits[i : i + limit], on_update=[]
                    )
                else:
                    nsi.on_wait = list(nsi.on_wait or []) + waits[i : i + limit]
        self.nc.all_engine_barrier()
        popped = self.nc._tile_sem_poison_stack.pop()
        assert popped is self._sem_poison
        self.nc.clear_and_free_semaphores(list(self.sems.allocated().values()))
        self.nc.all_engine_barrier()

    tile.TileContext._drain_and_barrier = _patched_dab
    tile.TileContext._drain_split_patched = True


_install_drain_split_patch()

P = 128
S = 2048
D = 4096
KO = D // P  # 32 contraction chunks
NH_L = 8  # q heads per core
NKV_L = 2  # kv heads per core
DH = 128
SCALE = float(DH) ** -0.5
N_CORES = 8

_dt_f32 = mybir.dt.float32
_dt_bf16 = mybir.dt.bfloat16
_EXP = mybir.ActivationFunctionType.Exp


def _emit(tc, aps):
    nc = tc.nc
    xr = aps["xT"].rearrange("(ko p) s -> p ko s", p=P)  # [128, 32, 2048]
    wqr = aps["wqT"].rearrange("(ko p) o -> p ko o", p=P)  # [128, 32, 1024]
    wkvr = aps["wkvT"].rearrange("(ko p) o -> p ko o", p=P)  # [128, 32, 512]
    wor = aps["woT"].rearrange("(ho p) e -> p ho e", p=P)  # [128, 8, 4096]
    out_ap = aps["out"]  # [2048, 4096] bf16
    cexp_ap = aps["cexp"]  # [128, 2048] bf16
    sexp_ap = aps["sexp"]

    from contextlib import ExitStack

    with ExitStack() as g:
        consts = g.enter_context(tc.tile_pool(name="consts", bufs=1))
        tabs = g.enter_context(tc.tile_pool(name="tabs", bufs=1))
        kt_pool = g.enter_context(tc.tile_pool(name="ktp", bufs=1))
        vt_pool = g.enter_context(tc.tile_pool(name="vtp", bufs=1))
        xt_pool = g.enter_context(tc.tile_pool(name="xtp", bufs=1))
        qt_pool = g.enter_context(tc.tile_pool(name="qtp", bufs=1))
        wq_pool = g.enter_context(tc.tile_pool(name="wqp", bufs=3))
        wkv_pool = g.enter_context(tc.tile_pool(name="wkvp", bufs=3))
        pt_pool = g.enter_context(tc.tile_pool(name="ptp", bufs=16))
        at_pool = g.enter_context(tc.tile_pool(name="atp", bufs=2))
        wo_pool = g.enter_context(tc.tile_pool(name="wop", bufs=2))
        l_pool = g.enter_context(tc.tile_pool(name="lp", bufs=1))
        ost_pool = g.enter_context(tc.tile_pool(name="ostp", bufs=2))
        tmp_pool = g.enter_context(tc.tile_pool(name="tmpp", bufs=2))
        # PSUM: 3 + 1 + 2 + 2 = 8 banks
        ps_sc = g.enter_context(tc.tile_pool(name="ps_sc", bufs=3, space="PSUM"))
        ps_sw = g.enter_context(tc.tile_pool(name="ps_sw", bufs=1, space="PSUM"))
        ps_acc = g.enter_context(tc.tile_pool(name="ps_acc", bufs=2, space="PSUM"))
        ps_pa = g.enter_context(tc.tile_pool(name="ps_pa", bufs=2, space="PSUM"))

        perm_t = consts.tile([P, P], _dt_bf16)
        maskT_t = consts.tile([P, P], _dt_f32)
        ones_t = consts.tile([P, P], _dt_bf16)
        nc.scalar.dma_start(out=perm_t, in_=aps["perm"])
        nc.scalar.dma_start(out=maskT_t, in_=aps["maskT"])
        nc.scalar.dma_start(out=ones_t, in_=aps["ones"])

        for wi in range(6):
            wup = ps_sc.tile([P, 512], _dt_f32, tag="sc")
            nc.tensor.matmul(
                wup[:, :P], lhsT=perm_t, rhs=perm_t, start=True, stop=True
            )

        kt_t = kt_pool.tile([P, NKV_L, S], _dt_bf16)  # K^T roped, full S
        v_t = vt_pool.tile([P, 16, NKV_L * DH], _dt_bf16)  # V natural, full S

        cp_flip = [0]

        def cp(out, in_):
            if cp_flip[0] % 2 == 0:
                nc.scalar.copy(out=out, in_=in_)
            else:
                nc.vector.tensor_copy(out=out, in_=in_)
            cp_flip[0] += 1

        proj_flip = [0]

        def proj_psum_tile():
            # proj chains alternate between the acc and (attention-idle) pa
            # rings for 4-deep chain pipelining; wo keeps acc exclusively so
            # PV never contends for pa slots during attention.
            proj_flip[0] += 1
            if proj_flip[0] % 2 == 0:
                gp = ps_acc.tile([P, 512], _dt_f32, tag="acc")
            else:
                gp = ps_pa.tile([P, 512], _dt_f32, tag="pa")
            return gp

        # ---- filler plumbing: closures emitting ~1 PE accumulation group ----
        fillers = []

        def drain(n):
            k = min(n, len(fillers))
            for _ in range(k):
                fillers.pop(0)()

        def drain_all():
            drain(len(fillers))

        def rope(dst, gsrc, ctab_t, stab_t, sc):
            """dst (sbuf bf16 [128,512]) <- rope(gsrc (psum f32 [128,512]))."""
            nc.scalar.copy(out=dst, in_=gsrc)
            sw = ps_sw.tile([P, 512], _dt_f32, tag="sw")
            nc.tensor.matmul(sw, lhsT=perm_t, rhs=dst, start=True, stop=True)
            t1 = tmp_pool.tile([P, 512], _dt_bf16, tag="t1")
            nc.vector.tensor_mul(t1, sw, stab_t[:, sc * 512 : (sc + 1) * 512])
            nc.vector.tensor_mul(dst, dst, ctab_t[:, sc * 512 : (sc + 1) * 512])
            nc.vector.tensor_add(dst, dst, t1)

        def v_group(xt_t, hi, sb_l, va, vb):
            sb = hi * 8 + sb_l
            gv = proj_psum_tile()
            for ko in range(KO):
                w = va if ko < 16 else vb
                nc.tensor.matmul(
                    gv[:, :256],
                    lhsT=xt_t[:, ko, sb_l * P : (sb_l + 1) * P],
                    rhs=w[:, ko % 16, :],
                    start=(ko == 0),
                    stop=(ko == KO - 1),
                )
            cp(v_t[:, sb, :], gv[:, :256])

        def k_group(xt_t, hi, sc, g2, ka, kb_, ctab_t, stab_t):
            gk = proj_psum_tile()
            for ko in range(KO):
                w = ka if ko < 16 else kb_
                nc.tensor.matmul(
                    gk,
                    lhsT=w[:, ko % 16, g2 * P : (g2 + 1) * P],
                    rhs=xt_t[:, ko, sc * 512 : (sc + 1) * 512],
                    start=(ko == 0),
                    stop=(ko == KO - 1),
                )
            rope(
                kt_t[:, g2, hi * 1024 + sc * 512 : hi * 1024 + (sc + 1) * 512],
                gk, ctab_t, stab_t, sc,
            )

        def q_group(xt_t, qt_t, hp, h2, sc, wa, wb, ctab_t, stab_t, rev=False):
            gq = proj_psum_tile()
            kos = range(KO - 1, -1, -1) if rev else range(KO)
            first = KO - 1 if rev else 0
            last = 0 if rev else KO - 1
            for ko in kos:
                w = wa if ko < 16 else wb
                nc.tensor.matmul(
                    gq,
                    lhsT=w[:, ko % 16, h2 * P : (h2 + 1) * P],
                    rhs=xt_t[:, ko, sc * 512 : (sc + 1) * 512],
                    start=(ko == first),
                    stop=(ko == last),
                )
            rope(
                qt_t[:, hp * 2 + h2, sc * 512 : (sc + 1) * 512],
                gq, ctab_t, stab_t, sc,
            )

        def attention(qc, qt_t):
            """S^T attention for 512-q chunk qc (global 0..3), all 8 heads.
            Fillers drained inside the kb loops to cover the exp deficit."""
            qc_l = qc & 1
            at_t = at_pool.tile([P, NH_L, 512], _dt_bf16, tag="at")
            nblk = 4 * qc + 4
            for h in range(NH_L):
                g2 = h // 4
                la = l_pool.tile([P, 512], _dt_bf16, tag="lacc_a")
                lb = l_pool.tile([P, 512], _dt_bf16, tag="lacc_b")
                slices = []
                for kb in range(nblk):
                    j = kb - 4 * qc
                    c0 = j * P if j > 0 else 0
                    ssc = ps_sc.tile([P, 512], _dt_f32, tag="sc")
                    nc.tensor.matmul(
                        ssc[:, c0:512],
                        lhsT=kt_t[:, g2, kb * P : (kb + 1) * P],
                        rhs=qt_t[:, h, qc_l * 512 + c0 : qc_l * 512 + 512],
                        start=True,
                        stop=True,
                    )
                    if j >= 0:
                        nc.vector.tensor_add(
                            ssc[:, c0 : c0 + P], ssc[:, c0 : c0 + P], maskT_t
                        )
                    p = pt_pool.tile([P, 512], _dt_bf16, tag="pt")
                    nc.scalar.activation(
                        out=p[:, c0:512], in_=ssc[:, c0:512], func=_EXP, scale=SCALE
                    )
                    if j >= 1:
                        nc.gpsimd.memset(p[:, 0:c0], 0.0)
                    slices.append(p)
                    # denominator partials interleaved with the exp stream so
                    # the DVE chain finishes with the chunk, not after it
                    if kb == 2:
                        nc.vector.tensor_add(la, slices[0], slices[2])
                    elif kb == 3:
                        nc.vector.tensor_add(lb, slices[1], slices[3])
                    elif kb >= 4:
                        acc = la if kb % 2 == 0 else lb
                        nc.vector.tensor_add(acc, acc, p)
                    if kb % 4 == 3:
                        drain(1)
                pa = ps_pa.tile([P, 512], _dt_f32, tag="pa")
                for kb in range(nblk):
                    nc.tensor.matmul(
                        pa,
                        lhsT=v_t[:, kb, g2 * P : (g2 + 1) * P],
                        rhs=slices[kb],
                        start=(kb == 0),
                        stop=(kb == nblk - 1),
                    )
                    if kb % 8 == 7:
                        drain(1)
                # combine denominator chains; one all-ones matmul does the
                # partition sum AND the broadcast (out[m,q] = sum_p la[p,q])
                nc.vector.tensor_add(la, la, lb)
                lsum = ps_sw.tile([P, 512], _dt_f32, tag="sw")
                nc.tensor.matmul(lsum, lhsT=ones_t, rhs=la, start=True, stop=True)
                linv = l_pool.tile([P, 512], _dt_f32, tag="linv")
                if qc < 2 or h == 7:
                    # early chunks: DVE is the pacer (little exp work, fast PE
                    # cadence) and its 6-cpe reciprocal backlogs the at-tile
                    # writes; compute linv = exp(-ln(lsum)) on the scalar
                    # engine instead, which has headroom here.
                    nl = l_pool.tile([P, 512], _dt_f32, tag="nl")
                    nc.scalar.activation(
                        out=nl, in_=lsum, func=mybir.ActivationFunctionType.Ln
                    )
                    nc.scalar.activation(out=linv, in_=nl, func=_EXP, scale=-1.0)
                else:
                    nc.vector.reciprocal(linv, lsum)
                nc.vector.tensor_mul(at_t[:, h, :], pa, linv)
            return at_t

        def wo_items(qc, at_t):
            """32 filler closures: wo GEMM for chunk qc, ec-outer/sb-inner so
            each streamed wo piece is reused by 4 s-blocks."""
            items = []
            state = {}
            for ec in range(8):
                for sb4 in range(4):
                    def f(ec=ec, sb4=sb4):
                        if sb4 == 0:
                            wp = wo_pool.tile([P, NH_L, 512], _dt_bf16, tag="wo")
                            nc.scalar.dma_start(
                                out=wp, in_=wor[:, :, ec * 512 : (ec + 1) * 512]
                            )
                            state[ec] = wp
                        wp = state[ec]
                        go = ps_acc.tile([P, 512], _dt_f32, tag="acc")
                        for hb in range(NH_L):
                            nc.tensor.matmul(
                                go,
                                lhsT=at_t[:, hb, sb4 * P : (sb4 + 1) * P],
                                rhs=wp[:, hb, :],
                                start=(hb == 0),
                                stop=(hb == NH_L - 1),
                            )
                        ost = ost_pool.tile([P, 512], _dt_bf16, tag="ost")
                        cp(ost, go)
                        nc.sync.dma_start(
                            out=out_ap[
                                qc * 512 + sb4 * P : qc * 512 + (sb4 + 1) * P,
                                ec * 512 : (ec + 1) * 512,
                            ],
                            in_=ost,
                        )
                    items.append(f)
            return items

        # ================= main schedule =================
        for hi in range(2):
            s0 = hi * 1024
            # rope tables for this half (scalar queue, tiny)
            ctab_t = tabs.tile([P, 1024], _dt_bf16, tag="ct")
            stab_t = tabs.tile([P, 1024], _dt_bf16, tag="st")
            nc.scalar.dma_start(out=ctab_t, in_=cexp_ap[:, s0 : s0 + 1024])
            nc.scalar.dma_start(out=stab_t, in_=sexp_ap[:, s0 : s0 + 1024])
            # V weights + first K piece on scalar queue (3 wkv slots live)
            va = wkv_pool.tile([P, 16, 256], _dt_bf16, tag="wkv")
            nc.scalar.dma_start(out=va, in_=wkvr[:, 0:16, 256:512])
            vb = wkv_pool.tile([P, 16, 256], _dt_bf16, tag="wkv")
            nc.scalar.dma_start(out=vb, in_=wkvr[:, 16:32, 256:512])
            ka = wkv_pool.tile([P, 16, 256], _dt_bf16, tag="wkv")
            nc.scalar.dma_start(out=ka, in_=wkvr[:, 0:16, 0:256])
            # x^T half on sync queue, s-ordered pieces
            xt_t = xt_pool.tile([P, KO, 1024], _dt_bf16, tag="xt")
            # first piece split fine so the very first V chain starts early
            for ci in range(2):
                c0, c1 = ci * 128, (ci + 1) * 128
                nc.sync.dma_start(
                    out=xt_t[:, 0:16, c0:c1], in_=xr[:, 0:16, s0 + c0 : s0 + c1]
                )
                nc.sync.dma_start(
                    out=xt_t[:, 16:32, c0:c1], in_=xr[:, 16:32, s0 + c0 : s0 + c1]
                )
            for pi in range(1, 4):
                nc.sync.dma_start(
                    out=xt_t[:, :, pi * 256 : (pi + 1) * 256],
                    in_=xr[:, :, s0 + pi * 256 : s0 + (pi + 1) * 256],
                )
            qt_t = qt_pool.tile([P, NH_L, 1024], _dt_bf16, tag="qt")

            # ---- projections: V sb0-7, K sc0+sc1, Q (all heads) ----
            for sb_l in range(8):
                v_group(xt_t, hi, sb_l, va, vb)
            kb_ = wkv_pool.tile([P, 16, 256], _dt_bf16, tag="wkv")
            nc.scalar.dma_start(out=kb_, in_=wkvr[:, 16:32, 0:256])
            for sc in range(2):
                for g2 in range(NKV_L):
                    k_group(xt_t, hi, sc, g2, ka, kb_, ctab_t, stab_t)
            for hp in range(4):
                wa = wq_pool.tile([P, 16, 256], _dt_bf16, tag="wq")
                nc.sync.dma_start(
                    out=wa, in_=wqr[:, 0:16, hp * 256 : (hp + 1) * 256]
                )
                wb = wq_pool.tile([P, 16, 256], _dt_bf16, tag="wq")
                nc.sync.dma_start(
                    out=wb, in_=wqr[:, 16:32, hp * 256 : (hp + 1) * 256]
                )
                # last group reads the 'wa' piece backwards so it frees early
                q_group(xt_t, qt_t, hp, 0, 0, wa, wb, ctab_t, stab_t)
                q_group(xt_t, qt_t, hp, 1, 0, wa, wb, ctab_t, stab_t)
                q_group(xt_t, qt_t, hp, 0, 1, wa, wb, ctab_t, stab_t)
                q_group(xt_t, qt_t, hp, 1, 1, wa, wb, ctab_t, stab_t, rev=True)

            # ---- attention qc_a = 2*hi, then qc_b = 2*hi+1 ----
            qc_a = 2 * hi
            at_a = attention(qc_a, qt_t)
            drain_all()
            fillers.extend(wo_items(qc_a, at_a))
            qc_b = 2 * hi + 1
            at_b = attention(qc_b, qt_t)
            # flush wo(qc_a) leftovers now: attention(qc_a+2) will reuse
            # at(qc_a)'s slot (at_pool bufs=2), so its readers must all be
            # emitted before then.
            drain_all()
            if hi == 0:
                # wo(qc1) items become the fillers for half-1 attn(qc2)
                fillers.extend(wo_items(qc_b, at_b))
            else:
                for f in wo_items(qc_b, at_b):
                    f()


def _split_excess_waits(nc, limit=1):
    """Walrus (pinned build) rejects instructions carrying more than ~2
    semaphore waits.  Hoist excess waits onto same-engine no-ops inserted
    immediately before the offending instruction."""
    ctr = [0]
    for bb in nc.main_func.blocks:
        insts = list(bb.instructions)
        out = []
        changed = False
        for ins in insts:
            si = ins.sync_info
            waits = list(si.on_wait) if si and si.on_wait else []
            if len(waits) > limit:
                keep = waits[:limit]
                rest = waits[limit:]
                for i in range(0, len(rest), limit):
                    nop = mybir.InstNoOp(name=f"I-waitsplit-{ctr[0]}", ins=[], outs=[])
                    ctr[0] += 1
                    nop.engine = ins.engine
                    nop.sync_info = mybir.SyncInfo(
                        on_wait=rest[i : i + limit], on_update=[]
                    )
                    nc.register_instruction(nop)
                    out.append(nop)
                si.on_wait = keep
                changed = True
            out.append(ins)
        if changed:
            bb.instructions = out
    return ctr[0]


_PROGRAM_CACHE = {}


def build_program():
    if "nc" in _PROGRAM_CACHE:
        return _PROGRAM_CACHE["nc"]
    nc = bass.Bass("TRN2", target_bir_lowering=False, debug=False, num_devices=N_CORES)
    aps = {
        "xT": nc.dram_tensor("xT", [D, S], _dt_bf16, kind="ExternalInput").ap(),
        "wqT": nc.dram_tensor("wqT", [D, NH_L * DH], _dt_bf16, kind="ExternalInput").ap(),
        "wkvT": nc.dram_tensor("wkvT", [D, 512], _dt_bf16, kind="ExternalInput").ap(),
        "woT": nc.dram_tensor("woT", [NH_L * DH, D], _dt_bf16, kind="ExternalInput").ap(),
        "cexp": nc.dram_tensor("cexp", [P, S], _dt_bf16, kind="ExternalInput").ap(),
        "sexp": nc.dram_tensor("sexp", [P, S], _dt_bf16, kind="ExternalInput").ap(),
        "perm": nc.dram_tensor("perm", [P, P], _dt_bf16, kind="ExternalInput").ap(),
        "maskT": nc.dram_tensor("maskT", [P, P], _dt_f32, kind="ExternalInput").ap(),
        "ones": nc.dram_tensor("ones", [P, P], _dt_bf16, kind="ExternalInput").ap(),
        "out": nc.dram_tensor("out", [S, D], _dt_bf16, kind="ExternalOutput").ap(),
    }
    with tile.TileContext(nc) as tc:
        _emit(tc, aps)
    _split_excess_waits(nc, limit=1)
    _PROGRAM_CACHE["nc"] = nc
    return nc


def make_in_maps(x, freqs_cos, freqs_sin, mask, wq, wk, wv, wo):
    x = np.asarray(x, np.float32)
    freqs_cos = np.asarray(freqs_cos, np.float32)
    freqs_sin = np.asarray(freqs_sin, np.float32)
    mask = np.asarray(mask, np.float32)
    wq = np.asarray(wq, np.float32)
    wk = np.asarray(wk, np.float32)
    wv = np.asarray(wv, np.float32)
    wo = np.asarray(wo, np.float32)

    xb = [np.ascontiguousarray(x[b].T).astype(BF16) for b in range(2)]  # (4096, 2048)
    cexp = np.repeat(freqs_cos.T, 2, axis=0).astype(BF16)  # (128, 2048)
    sx = np.repeat(freqs_sin.T, 2, axis=0).astype(np.float32)
    sx[0::2] *= -1.0
    sexp = sx.astype(BF16)
    perm = np.zeros((P, P), np.float32)
    idx = np.arange(P)
    perm[idx, idx ^ 1] = 1.0
    perm = perm.astype(BF16)
    maskT = np.ascontiguousarray(mask[:P, :P].T, dtype=np.float32)
    ones = np.ones((P, P), dtype=BF16)

    in_maps = []
    for core in range(N_CORES):
        b, tp = core // 4, core % 4
        wqT = np.ascontiguousarray(wq[tp * 1024 : (tp + 1) * 1024].T).astype(BF16)
        wkT = wk[tp * 256 : (tp + 1) * 256].T.astype(BF16)  # (4096, 256)
        wvT = wv[tp * 256 : (tp + 1) * 256].T.astype(BF16)
        wkvT = np.ascontiguousarray(np.concatenate([wkT, wvT], axis=1))
        woT = np.ascontiguousarray(wo[:, tp * 1024 : (tp + 1) * 1024].T).astype(BF16)
        in_maps.append(
            {
                "xT": xb[b],
                "wqT": wqT,
                "wkvT": wkvT,
                "woT": woT,
                "cexp": cexp,
                "sexp": sexp,
                "perm": perm,
                "maskT": maskT,
                "ones": ones,
            }
        )
    return in_maps


def run(inputs, trace=False):
    nc = build_program()
    in_maps = make_in_maps(
        inputs["x"],
        inputs["freqs_cos"],
        inputs["freqs_sin"],
        inputs["mask"],
        inputs["wq"],
        inputs["wk"],
        inputs["wv"],
        inputs["wo"],
    )
    res = run_bass_kernel_spmd(nc, in_maps, list(range(N_CORES)), trace=trace)
    out = np.zeros((2, S, D), np.float32)
    for core in range(N_CORES):
        out[core // 4] += np.asarray(res.results[core]["out"]).astype(np.float32)
    return out, res


def kernel(x, freqs_cos, freqs_sin, positions, mask, wq, wk, wv, wo):
    out, _ = run(
        {
            "x": x,
            "freqs_cos": freqs_cos,
            "freqs_sin": freqs_sin,
            "mask": mask,
            "wq": wq,
            "wk": wk,
            "wv": wv,
            "wo": wo,
        }
    )
    return out
